# revision 1
# baseline (speedup 1.0000x reference)
"""Expert-parallel MoE FFN kernel for Trainium2 (8 NeuronCores, one expert per core).

Host side: routes tokens to experts (dedup per expert, summing duplicate top-k
weights), pads each expert's token list to a common T_PAD, and pre-tiles the
weight matrices into DMA-friendly contiguous blocks.

Device side (per core, expert e):
  h^T = silu(G_e^T X^T) * (U_e^T X^T)        [I, T]   (stage A, fp32r matmuls)
  y   = (h^T)^T-contracted-with D_e * cw      [T, H]   (stage B)
All matmuls run as float32r (tf32-like rounding, 1 cycle/row on the PE vs 4
for plain fp32); accumulation is fp32 in PSUM.
"""
import sys

if "/opt/trn_rl_repo" not in sys.path:
    sys.path.insert(0, "/opt/trn_rl_repo")

import numpy as np

N_TOKENS, TOP_K, N_EXPERTS, HIDDEN, INTER = 4096, 2, 8, 1024, 2048
P = 128
NI = INTER // P          # 16 I-tiles
KH = HIDDEN // P         # 8 H(contraction)-tiles
HC = HIDDEN // 512       # 2 output-column chunks

_CACHE = {}
MM_BF16 = True


def _build(t_pad):
    import concourse.bacc as bacc
    import concourse.mybir as mybir
    import concourse.tile as tile

    f32 = mybir.dt.float32
    f32r = mybir.dt.bfloat16 if MM_BF16 else mybir.dt.float32r

    nt = t_pad // P          # T tiles of 128
    ntc = t_pad // 512       # T chunks of 512

    nc = bacc.Bacc()
    xt = nc.declare_dram_parameter("xt", [KH, P, t_pad], f32r, isOutput=False)
    gw = nc.declare_dram_parameter("gw", [NI, P, HIDDEN], f32r, isOutput=False)
    uw = nc.declare_dram_parameter("uw", [NI, P, HIDDEN], f32r, isOutput=False)
    dw = nc.declare_dram_parameter("dw", [NI, P, HIDDEN], f32r, isOutput=False)
    cw = nc.declare_dram_parameter("cw", [P, t_pad], f32, isOutput=False)
    y = nc.declare_dram_parameter("y", [HIDDEN, t_pad], f32, isOutput=True)

    with tile.TileContext(nc) as tc:
        with (
            tc.tile_pool(name="hp", bufs=1) as hp,
            tc.tile_pool(name="wp", bufs=2) as wp,
            tc.tile_pool(name="ep", bufs=3) as ep,
            tc.tile_pool(name="cp", bufs=1) as cp,
        ):
            cwt = cp.tile([P, t_pad], f32)
            nc.sync.dma_start(out=cwt[:], in_=cw[:])

            hts = [hp.tile([P, t_pad], f32r, tag=f"h{i}", name=f"ht{i}") for i in range(NI)]

            # ---- Stage A: h^T[i] = silu(G^T X^T) * (U^T X^T), tiled over I ----
            with (
                tc.tile_pool(name="xp", bufs=1) as xp,
                tc.tile_pool(name="psA", bufs=2, space="PSUM") as psA,
            ):
                xts = []
                for k in range(KH):
                    t = xp.tile([P, t_pad], f32r, tag=f"x{k}")
                    nc.sync.dma_start(out=t[:], in_=xt[k])
                    xts.append(t)
                for i in range(NI):
                    gt = wp.tile([P, HIDDEN], f32r, tag="g")
                    ut = wp.tile([P, HIDDEN], f32r, tag="u")
                    nc.sync.dma_start(out=gt[:], in_=gw[i])
                    nc.sync.dma_start(out=ut[:], in_=uw[i])
                    pgs = [psA.tile([P, 512], f32, tag=f"pg{c}", name=f"pg{i}_{c}") for c in range(ntc)]
                    pus = [psA.tile([P, 512], f32, tag=f"pu{c}", name=f"pu{i}_{c}") for c in range(ntc)]
                    for k in range(KH):
                        lg = gt[:, k * P:(k + 1) * P]
                        lu = ut[:, k * P:(k + 1) * P]
                        for c in range(ntc):
                            rx = xts[k][:, c * 512:(c + 1) * 512]
                            nc.tensor.matmul(out=pgs[c][:], lhsT=lg, rhs=rx,
                                             start=(k == 0), stop=(k == KH - 1))
                        for c in range(ntc):
                            rx = xts[k][:, c * 512:(c + 1) * 512]
                            nc.tensor.matmul(out=pus[c][:], lhsT=lu, rhs=rx,
                                             start=(k == 0), stop=(k == KH - 1))
                    for c in range(ntc):
                        sg = ep.tile([P, 512], f32, tag="sg")
                        nc.scalar.activation(out=sg[:], in_=pgs[c][:],
                                             func=mybir.ActivationFunctionType.Silu)
                        nc.vector.tensor_mul(out=hts[i][:, c * 512:(c + 1) * 512],
                                             in0=sg[:], in1=pus[c][:])

            # ---- Stage B: y^T[j,:] = sum_i D[i,j-cols]^T @ h^T[i], * cw ----
            # dw tile is the stationary operand: one weight load serves ntc
            # matmuls. Output is y^T [H, T]; host transposes back.
            jg = max(1, 8 // ntc)          # j-tiles per group, jg*ntc <= 8 banks
            with (
                tc.tile_pool(name="dwp", bufs=1) as dwp,
                tc.tile_pool(name="psB", bufs=1, space="PSUM") as psB,
            ):
                dts = []
                for i in range(NI):
                    dt_ = dwp.tile([P, HIDDEN], f32r, tag=f"d{i}", name=f"dt{i}")
                    nc.sync.dma_start(out=dt_[:], in_=dw[i])
                    dts.append(dt_)
                for j0 in range(0, KH, jg):
                    pys = [psB.tile([P, 512], f32, tag=f"py{jj}_{c}",
                                    name=f"py{j0}_{jj}_{c}")
                           for jj in range(jg) for c in range(ntc)]
                    for i in range(NI):
                        for jj in range(jg):
                            ld = dts[i][:, (j0 + jj) * P:(j0 + jj + 1) * P]
                            for c in range(ntc):
                                nc.tensor.matmul(out=pys[jj * ntc + c][:],
                                                 lhsT=ld,
                                                 rhs=hts[i][:, c * 512:(c + 1) * 512],
                                                 start=(i == 0), stop=(i == NI - 1))
                    for jj in range(jg):
                        for c in range(ntc):
                            ysb = ep.tile([P, 512], f32, tag="y")
                            nc.vector.tensor_mul(out=ysb[:],
                                                 in0=pys[jj * ntc + c][:],
                                                 in1=cwt[:, c * 512:(c + 1) * 512])
                            nc.gpsimd.dma_start(
                                out=y[(j0 + jj) * P:(j0 + jj + 1) * P,
                                      c * 512:(c + 1) * 512],
                                in_=ysb[:])

    nc.finalize()
    return nc


def _route(expert_indices, expert_weights):
    idx = np.asarray(expert_indices).astype(np.int64)
    wts = np.asarray(expert_weights).astype(np.float32)
    n = idx.shape[0]
    cw_full = np.zeros((N_EXPERTS, n), np.float32)
    for k in range(idx.shape[1]):
        np.add.at(cw_full, (idx[:, k], np.arange(n)), wts[:, k])
    ids = [np.nonzero(cw_full[e])[0] for e in range(N_EXPERTS)]
    maxc = max(len(i) for i in ids)
    t_pad = max(512, ((maxc + 511) // 512) * 512)
    return cw_full, ids, t_pad


_LDW_PATCHED = False


def _patch_ldw_opt():
    """Enable walrus's LDWEIGHTS dedup pass: consecutive matmuls that reuse the
    same stationary tile then skip the redundant ~190ns weight reload."""
    global _LDW_PATCHED
    if _LDW_PATCHED:
        return
    import concourse.bass_utils as bu

    orig = bu.run_command

    def run_command(argv, **kw):
        argv = ["--enable-ldw-opt=true" if a == "--enable-ldw-opt=false" else a
                for a in argv]
        return orig(argv, **kw)

    bu.run_command = run_command
    _LDW_PATCHED = True


def _run(nc, in_maps, trace=False, trace_cores=None):
    from concourse.bass_utils import run_bass_kernel_spmd

    return run_bass_kernel_spmd(
        nc, in_maps, list(range(N_EXPERTS)), trace=trace,
        trace_cores=trace_cores,
    )


def prepare(tokens, expert_indices, expert_weights, gate_weight, up_weight,
            down_weight):
    """Host-side routing + layout. Returns (nc, in_maps, ids, t_pad)."""
    tokens = np.ascontiguousarray(np.asarray(tokens, dtype=np.float32))
    gate_weight = np.asarray(gate_weight, dtype=np.float32)
    up_weight = np.asarray(up_weight, dtype=np.float32)
    down_weight = np.asarray(down_weight, dtype=np.float32)

    cw_full, ids, t_pad = _route(expert_indices, expert_weights)
    nt = t_pad // P

    key = t_pad
    if key not in _CACHE:
        _CACHE[key] = _build(t_pad)
    nc = _CACHE[key]

    in_maps = []
    for e in range(N_EXPERTS):
        ce = len(ids[e])
        xe = np.zeros((HIDDEN, t_pad), np.float32)
        xe[:, :ce] = tokens[ids[e]].T
        cwe = np.zeros((t_pad,), np.float32)
        cwe[:ce] = cw_full[e, ids[e]]
        mmdt = np.dtype("bfloat16") if MM_BF16 else np.float32
        in_maps.append({
            "xt": np.ascontiguousarray(xe.reshape(KH, P, t_pad)).astype(mmdt),
            "gw": np.ascontiguousarray(
                gate_weight[e].reshape(KH, P, NI, P).transpose(2, 1, 0, 3)
            ).reshape(NI, P, HIDDEN).astype(mmdt),
            "uw": np.ascontiguousarray(
                up_weight[e].reshape(KH, P, NI, P).transpose(2, 1, 0, 3)
            ).reshape(NI, P, HIDDEN).astype(mmdt),
            "dw": np.ascontiguousarray(down_weight[e].reshape(NI, P, HIDDEN)).astype(mmdt),
            "cw": np.ascontiguousarray(
                np.broadcast_to(cwe[None, :], (P, t_pad))),
        })
    return nc, in_maps, ids, t_pad


def combine(results, ids):
    out = np.zeros((N_TOKENS, HIDDEN), np.float32)
    for e in range(N_EXPERTS):
        ce = len(ids[e])
        out[ids[e]] += results[e]["y"].T[:ce]
    return out


def kernel(tokens, expert_indices, expert_weights, gate_weight, up_weight,
           down_weight):
    nc, in_maps, ids, _ = prepare(tokens, expert_indices, expert_weights,
                                  gate_weight, up_weight, down_weight)
    res = _run(nc, in_maps, trace=False)
    return combine(res.results, ids)



# revision 4
# speedup vs baseline: 1.1313x; 1.1313x over previous
"""Expert-parallel MoE FFN kernel for Trainium2 (8 NeuronCores, one expert per core).

Host side: routes tokens to experts (dedup per expert, summing duplicate top-k
weights), pads each expert's token list to the max expert count t_pad (NOT
rounded to 512 — matmul free dim is arbitrary <=512), and pre-tiles the weight
matrices into DMA-friendly contiguous blocks.

Device side (per core, expert e):
  h^T = silu(G_e^T X^T) * (U_e^T X^T)        [I, T]   (stage A)
  y^T = (D^T h^T) * cw                        [H, T]   (stage B)
All matmuls in bf16 (1 col/cycle on the PE), fp32 accumulation in PSUM.

Perf-critical structure (from NTFF trace analysis of the previous version):
 - Input DMAs are spread across engine queues (sync: X, scalar: gate W,
   vector: up W) so the serialized ~0.6us-per-DMA issue cost doesn't gate the
   first matmul (was 18us to first MM on a single queue).
 - ~24 warmup matmuls on a memset tile run during the initial DMA wait so the
   PE HAM clock-gate reaches 2.4 GHz before real matmuls start.
 - PSUM is hand-scheduled as 8 explicit bank tiles. Stage A double-buffers
   (even i -> banks 0-3, odd i -> 4-7); stage B's first accumulator group
   reuses banks 0-3 (freed mid stage A) so the tensor engine never idles at
   the A->B transition (previous version lost ~5us to a HAM re-throttle there).
 - Stage B's accumulators are grouped <=4 banks; the last two groups are
   single (jj,c) atoms so the post-last-matmul tail is one mul + one DMA.
 - down-proj W DMAs are issued mid stage A from the scalar queue so their 4MB
   doesn't compete with the startup-critical X/G/U transfers.
"""
import sys

if "/opt/trn_rl_repo" not in sys.path:
    sys.path.insert(0, "/opt/trn_rl_repo")

import numpy as np

N_TOKENS, TOP_K, N_EXPERTS, HIDDEN, INTER = 4096, 2, 8, 1024, 2048
P = 128
NI = INTER // P          # 16 I-tiles
KH = HIDDEN // P         # 8 H(contraction)-tiles
N_WARMUP = 24
PREFETCH = 4

_CACHE = {}


def _chunks_of(tp):
    out = []
    off = 0
    while off < tp:
        sz = min(512, tp - off)
        out.append((off, sz))
        off += sz
    return out


def _build(t_pad):
    import concourse.bacc as bacc
    import concourse.mybir as mybir
    import concourse.tile as tile

    f32 = mybir.dt.float32
    bf16 = mybir.dt.bfloat16
    tp = t_pad
    chunks = _chunks_of(tp)
    ntc = len(chunks)
    assert tp <= 2048, f"t_pad {tp} unsupported"

    nc = bacc.Bacc()
    xt = nc.declare_dram_parameter("xt", [P, KH * tp], bf16, isOutput=False)
    gw = nc.declare_dram_parameter("gw", [NI, P, HIDDEN], bf16, isOutput=False)
    uw = nc.declare_dram_parameter("uw", [NI, P, HIDDEN], bf16, isOutput=False)
    dw = nc.declare_dram_parameter("dw", [NI // 4, P, 4 * HIDDEN], bf16,
                                   isOutput=False)
    cw = nc.declare_dram_parameter("cw", [P, tp], f32, isOutput=False)
    y = nc.declare_dram_parameter("y", [HIDDEN, tp], f32, isOutput=True)

    with tile.TileContext(nc) as tc:
        with (
            tc.tile_pool(name="ps", bufs=1, space="PSUM") as ps,
            tc.tile_pool(name="sb", bufs=1) as sb,
            tc.tile_pool(name="sm", bufs=2) as sm,
        ):
            # --- startup: warmup source + X on sync, first G/U tiles on
            # scalar/vector ---
            wsrc = sb.tile([P, P], bf16, tag="wsrc", name="wsrc")
            nc.gpsimd.memset(wsrc[:], 0.0)

            # Startup-critical tiles land first, one per DMA-capable queue:
            # sync: X(k=0), scalar: G(0), gpsimd: U(0). Everything else
            # streams on sync in deadline order — per-queue in-order transfer
            # paces it so the early X chunks aren't starved by weight bytes.
            xa = sb.tile([P, tp], bf16, tag="xa", name="xa")
            xb = sb.tile([P, 3 * tp], bf16, tag="xb", name="xb")
            xc = sb.tile([P, 4 * tp], bf16, tag="xc", name="xc")
            nc.sync.dma_start(out=xa[:], in_=xt[:, 0:tp])

            def xk(k):
                if k == 0:
                    return xa
                if k < 4:
                    return xb[:, (k - 1) * tp:k * tp]
                return xc[:, (k - 4) * tp:(k - 3) * tp]

            gts = [None] * NI
            uts = [None] * NI

            def issue_g(i, eng):
                gts[i] = sb.tile([P, HIDDEN], bf16, tag=f"g{i}", name=f"g{i}")
                eng.dma_start(out=gts[i][:], in_=gw[i])

            def issue_u(i, eng):
                uts[i] = sb.tile([P, HIDDEN], bf16, tag=f"u{i}", name=f"u{i}")
                eng.dma_start(out=uts[i][:], in_=uw[i])

            issue_g(0, nc.scalar)
            issue_u(0, nc.gpsimd)
            nc.sync.dma_start(out=xb[:], in_=xt[:, tp:4 * tp])
            nc.sync.dma_start(out=xc[:], in_=xt[:, 4 * tp:8 * tp])
            for i in range(1, NI):
                issue_g(i, nc.sync)
                issue_u(i, nc.sync)

            dts = [None] * 4
            for q in range(4):
                dts[q] = sb.tile([P, 4 * HIDDEN], bf16, tag=f"dw{q}",
                                 name=f"dwt{q}")
                nc.sync.dma_start(out=dts[q][:], in_=dw[q])

            def dslice(i, jj):
                q, r = divmod(i, 4)
                return dts[q][:, r * HIDDEN + jj * P: r * HIDDEN + (jj + 1) * P]

            cwt = sb.tile([P, tp], f32, tag="cw", name="cwt")
            nc.sync.dma_start(out=cwt[:], in_=cw[:])

            def bank(idx, name):
                return ps.tile([P, 512], f32, tag=f"b{idx}", name=name)

            # --- PE warmup: ~24 N=128 matmuls on the memset tile keep the PE
            # busy during the input DMA wait so HAM un-throttles early ---
            wps = bank(7, "warm_ps")
            for r in range(N_WARMUP):
                nc.tensor.matmul(out=wps[:, 0:P], lhsT=wsrc[:], rhs=wsrc[:],
                                 start=True, stop=True)

            hts = [sb.tile([P, tp], bf16, tag=f"h{i}", name=f"ht{i}")
                   for i in range(NI)]

            # ---- Stage A: h^T[i] = silu(G^T X^T) * (U^T X^T) ----
            for i in range(NI):
                s = (i % 2) * 4 if ntc <= 2 else 0
                pg = [bank(s + c, f"pg{i}_{c}") for c in range(ntc)]
                pu = [bank(s + ntc + c, f"pu{i}_{c}") for c in range(ntc)]
                gt, ut = gts[i], uts[i]
                for k in range(KH):
                    lg = gt[:, k * P:(k + 1) * P]
                    lu = ut[:, k * P:(k + 1) * P]
                    xv = xk(k)
                    for c, (o, szc) in enumerate(chunks):
                        nc.tensor.matmul(out=pg[c][:, 0:szc], lhsT=lg,
                                         rhs=xv[:, o:o + szc],
                                         start=(k == 0), stop=(k == KH - 1))
                    for c, (o, szc) in enumerate(chunks):
                        nc.tensor.matmul(out=pu[c][:, 0:szc], lhsT=lu,
                                         rhs=xv[:, o:o + szc],
                                         start=(k == 0), stop=(k == KH - 1))
                for c, (o, szc) in enumerate(chunks):
                    sg = sm.tile([P, 512], f32, tag=f"sg{c}", name=f"sg{i}_{c}")
                    nc.scalar.activation(out=sg[:, 0:szc], in_=pg[c][:, 0:szc],
                                         func=mybir.ActivationFunctionType.Silu)
                    nc.vector.tensor_mul(out=hts[i][:, o:o + szc],
                                         in0=sg[:, 0:szc], in1=pu[c][:, 0:szc])


            # ---- Stage B: y^T[jj] = sum_i D[i,jj]^T @ h^T[i], * cw ----
            # atoms = accumulator banks (jj, chunk); grouped <=4 banks, last
            # two groups are singletons to minimize the post-matmul tail.
            atoms = [(jj, c) for jj in range(KH) for c in range(ntc)]
            groups = []
            rest = atoms
            while len(rest) > 4:
                groups.append(rest[:4])
                rest = rest[4:]
            if len(rest) == 4:
                groups += [rest[:2], rest[2:3], rest[3:4]]
            elif len(rest) == 3:
                groups += [rest[:1], rest[1:2], rest[2:3]]
            else:
                groups += [rest[:1], rest[1:2]] if len(rest) == 2 else [rest]

            side = 0
            for g, grp in enumerate(groups):
                base = side * 4
                pys = {}
                for a, (jj, c) in enumerate(grp):
                    pys[(jj, c)] = bank(base + a, f"py{jj}_{c}")
                for i in range(NI):
                    last_jj = None
                    for (jj, c) in grp:
                        if jj != last_jj:
                            ld = dslice(i, jj)
                            last_jj = jj
                        o, szc = chunks[c]
                        nc.tensor.matmul(out=pys[(jj, c)][:, 0:szc], lhsT=ld,
                                         rhs=hts[i][:, o:o + szc],
                                         start=(i == 0), stop=(i == NI - 1))
                for a, (jj, c) in enumerate(grp):
                    o, szc = chunks[c]
                    yb = sm.tile([P, 512], f32, tag=f"yb{a}", name=f"yb{jj}_{c}")
                    nc.vector.tensor_mul(out=yb[:, 0:szc],
                                         in0=pys[(jj, c)][:, 0:szc],
                                         in1=cwt[:, o:o + szc])
                    eng = nc.gpsimd if (jj * ntc + c) % 2 else nc.sync
                    eng.dma_start(out=y[jj * P:(jj + 1) * P, o:o + szc],
                                  in_=yb[:, 0:szc])
                side ^= 1

    nc.finalize()
    return nc


def _route(expert_indices, expert_weights):
    idx = np.asarray(expert_indices).astype(np.int64)
    wts = np.asarray(expert_weights).astype(np.float32)
    n = idx.shape[0]
    cw_full = np.zeros((N_EXPERTS, n), np.float32)
    for k in range(idx.shape[1]):
        np.add.at(cw_full, (idx[:, k], np.arange(n)), wts[:, k])
    ids = [np.nonzero(cw_full[e])[0] for e in range(N_EXPERTS)]
    maxc = max(len(i) for i in ids)
    t_pad = max(512, ((maxc + 7) // 8) * 8)
    return cw_full, ids, t_pad


def _run(nc, in_maps, trace=False, trace_cores=None):
    from concourse.bass_utils import run_bass_kernel_spmd

    return run_bass_kernel_spmd(
        nc, in_maps, list(range(N_EXPERTS)), trace=trace,
        trace_cores=trace_cores,
    )


def prepare(tokens, expert_indices, expert_weights, gate_weight, up_weight,
            down_weight):
    """Host-side routing + layout. Returns (nc, in_maps, ids, t_pad)."""
    tokens = np.ascontiguousarray(np.asarray(tokens, dtype=np.float32))
    gate_weight = np.asarray(gate_weight, dtype=np.float32)
    up_weight = np.asarray(up_weight, dtype=np.float32)
    down_weight = np.asarray(down_weight, dtype=np.float32)

    cw_full, ids, t_pad = _route(expert_indices, expert_weights)

    key = t_pad
    if key not in _CACHE:
        _CACHE[key] = _build(t_pad)
    nc = _CACHE[key]

    bf16 = np.dtype("bfloat16")
    in_maps = []
    for e in range(N_EXPERTS):
        ce = len(ids[e])
        xe = np.zeros((HIDDEN, t_pad), np.float32)
        xe[:, :ce] = tokens[ids[e]].T
        cwe = np.zeros((t_pad,), np.float32)
        cwe[:ce] = cw_full[e, ids[e]]
        in_maps.append({
            # [P, KH*tp]: xt[p, k*tp+t] = X^T[k*128+p, t]
            "xt": np.ascontiguousarray(
                xe.reshape(KH, P, t_pad).transpose(1, 0, 2)
            ).reshape(P, KH * t_pad).astype(bf16),
            # [NI, P, HIDDEN]: gw[i][p][k*128+q] = G[k*128+p, i*128+q]
            "gw": np.ascontiguousarray(
                gate_weight[e].reshape(KH, P, NI, P).transpose(2, 1, 0, 3)
            ).reshape(NI, P, HIDDEN).astype(bf16),
            "uw": np.ascontiguousarray(
                up_weight[e].reshape(KH, P, NI, P).transpose(2, 1, 0, 3)
            ).reshape(NI, P, HIDDEN).astype(bf16),
            # [4, P, 4*HIDDEN]: dw[q][p][r*H+h] = D[(4q+r)*128+p, h]
            "dw": np.ascontiguousarray(
                down_weight[e].reshape(4, 4, P, HIDDEN).transpose(0, 2, 1, 3)
            ).reshape(4, P, 4 * HIDDEN).astype(bf16),
            "cw": np.ascontiguousarray(
                np.broadcast_to(cwe[None, :], (P, t_pad))),
        })
    return nc, in_maps, ids, t_pad


def combine(results, ids):
    out = np.zeros((N_TOKENS, HIDDEN), np.float32)
    for e in range(N_EXPERTS):
        ce = len(ids[e])
        out[ids[e]] += results[e]["y"].T[:ce]
    return out


def kernel(tokens, expert_indices, expert_weights, gate_weight, up_weight,
           down_weight):
    nc, in_maps, ids, _ = prepare(tokens, expert_indices, expert_weights,
                                  gate_weight, up_weight, down_weight)
    res = _run(nc, in_maps, trace=False)
    return combine(res.results, ids)
